# revision 1
# baseline (speedup 1.0000x reference)
"""MoE MLP (E=4, top-2 routing) Trainium2 kernel, 8 NeuronCores.

Strategy ("pair-group" sharding): tokens are grouped on the host by their
routed expert PAIR (6 possible pairs for E=4).  Each of the 8 cores gets one
contiguous window of tokens that all share the same expert pair (a, b), plus
the full weights of those two experts.  Each core computes
    z = p_a * gelu(x @ w1[a]) @ w2[a] + p_b * gelu(x @ w1[b]) @ w2[b] + res
for its window — entirely locally, so no collectives are needed.  The host
only permutes rows back to token order afterwards (no arithmetic on the
common path).

Tokens with !=2 routed experts are decomposed into "virtual rows" of <=2
contributions each; if the resulting group structure does not fit 8 windows
(non-top-2 routing), a dense fallback (every core: 256 tokens x all 4
experts) is used.
"""
import math
import sys

import numpy as np

try:
    import concourse.bass as bass  # noqa: F401
except Exception:
    sys.path.insert(0, "/opt/trn_rl_repo")

import concourse.bacc as bacc
import concourse.bass as bass
import concourse.mybir as mybir
import concourse.tile as tile
from concourse.bass_utils import run_bass_kernel_spmd

S, B, H, F, E = 1024, 2, 1024, 4096, 4
T = S * B
N_CORES = 8
NH = H // 128   # 8
NF = F // 128   # 32
MM_DT = mybir.dt.float16  # full PE rate, ~2^-11 operand rounding
MM_NP = np.float16


def _plan_windows(routing_map, probs):
    """Decompose tokens into virtual rows and pack them into 8 pure windows.

    Returns (n_slots, C, windows) where windows is a list of 8 tuples
    (experts_tuple, vrow_list); each vrow is (t, pa, pb, first).
    """
    groups = {}
    for t in range(T):
        es = np.nonzero(routing_map[t])[0]
        if len(es) == 0:
            groups.setdefault((0, 0), []).append((t, 0.0, 0.0, True))
        else:
            for k in range(0, len(es), 2):
                pair = es[k : k + 2]
                if len(pair) == 1:
                    a = b = int(pair[0])
                    pa, pb = float(probs[t, a]), 0.0
                else:
                    a, b = int(pair[0]), int(pair[1])
                    pa, pb = float(probs[t, a]), float(probs[t, b])
                groups.setdefault((a, b), []).append((t, pa, pb, k == 0))

    for C in (128, 256, 384, 512):
        if sum(math.ceil(len(g) / C) for g in groups.values()) <= N_CORES:
            windows = []
            for (a, b), lst in sorted(groups.items()):
                nparts = math.ceil(len(lst) / C)
                step = math.ceil(len(lst) / nparts)
                for i in range(nparts):
                    windows.append(((a, b), lst[i * step : (i + 1) * step]))
            while len(windows) < N_CORES:
                windows.append(((0, 0), []))
            return 2, C, windows
    # dense fallback: all 4 experts on every core, 256 tokens per core
    C = T // N_CORES
    windows = []
    for c in range(N_CORES):
        lst = [(t, 0.0, 0.0, True) for t in range(c * C, (c + 1) * C)]
        windows.append(((0, 1, 2, 3), lst))
    return E, C, windows


_NC_CACHE = {}


def _build_nc(n_slots, C):
    key = (n_slots, C)
    if key in _NC_CACHE:
        return _NC_CACHE[key]
    NT = C // 128
    f32 = mybir.dt.float32
    nc = bacc.Bacc("TRN2", target_bir_lowering=False, debug=False,
                   num_devices=N_CORES)
    xt_d = nc.declare_dram_parameter("xt", [H, C], MM_DT, isOutput=False)
    w1_d = nc.declare_dram_parameter("w1b", [n_slots, NF, 128, H], MM_DT,
                                     isOutput=False)
    w2_d = nc.declare_dram_parameter("w2b", [n_slots, F, H], MM_DT,
                                     isOutput=False)
    pp_d = nc.declare_dram_parameter("pp", [n_slots, C], f32, isOutput=False)
    res_d = nc.declare_dram_parameter("res", [C, H], f32, isOutput=False)
    out_d = nc.declare_dram_parameter("out", [C, H], f32, isOutput=True)

    with tile.TileContext(nc) as tc:
        with (
            tc.tile_pool(name="resident", bufs=1) as rpool,
            tc.tile_pool(name="w1", bufs=8) as w1pool,
            tc.tile_pool(name="w2", bufs=12) as w2pool,
            tc.tile_pool(name="abig", bufs=2) as apool,
            tc.tile_pool(name="tmp", bufs=4) as tpool,
            tc.tile_pool(name="pa", bufs=3, space="PSUM") as papool,
            tc.tile_pool(name="py", bufs=NT, space="PSUM") as pypool,
        ):
            xt_sb = rpool.tile([128, NH, C], MM_DT, tag="xt")
            nc.sync.dma_start(
                xt_sb[:], xt_d.ap().rearrange("(hc h) c -> h hc c", h=128))
            res_sb = rpool.tile([128, NT, H], f32, tag="res")
            nc.sync.dma_start(
                res_sb[:], res_d.ap().rearrange("(tc t) d -> t tc d", t=128))
            pp_sb = rpool.tile([128, n_slots, NT], f32, tag="pp")
            nc.sync.dma_start(
                pp_sb[:], pp_d.ap().rearrange("s (tc t) -> t s tc", t=128))
            z_sb = rpool.tile([128, NT, H], f32, tag="z")

            for s in range(n_slots):
                a_big = apool.tile([128, NF, C], MM_DT, tag="a")
                for Fc in range(NF):
                    w1t = w1pool.tile([128, H], MM_DT, tag="w1")
                    nc.sync.dma_start(w1t[:], w1_d[s, Fc])
                    pa = papool.tile([128, C], f32, tag="pa")
                    for Hc in range(NH):
                        nc.tensor.matmul(
                            pa[:, :],
                            w1t[:, Hc * 128:(Hc + 1) * 128],
                            xt_sb[:, Hc, :],
                            start=(Hc == 0), stop=(Hc == NH - 1))
                    nc.scalar.activation(
                        a_big[:, Fc, :], pa[:, :],
                        mybir.ActivationFunctionType.Gelu)
                for Hh in range(2):
                    psum_ys = [pypool.tile([128, 512], f32, tag="py",
                                           name=f"py_{s}_{Hh}_{i}")
                               for i in range(NT)]
                    for Fc in range(NF):
                        w2t = w2pool.tile([128, 512], MM_DT, tag="w2")
                        nc.sync.dma_start(
                            w2t[:],
                            w2_d[s, Fc * 128:(Fc + 1) * 128,
                                 Hh * 512:(Hh + 1) * 512])
                        for Tc in range(NT):
                            nc.tensor.matmul(
                                psum_ys[Tc][:, :],
                                a_big[:, Fc,
                                      Tc * 128:(Tc + 1) * 128],
                                w2t[:, :],
                                start=(Fc == 0), stop=(Fc == NF - 1))
                    for Tc in range(NT):
                        zsl = z_sb[:, Tc, Hh * 512:(Hh + 1) * 512]
                        pcol = pp_sb[:, s, Tc:Tc + 1]
                        if s == 0:
                            nc.vector.tensor_scalar(
                                zsl, psum_ys[Tc][:, :], pcol, None,
                                mybir.AluOpType.mult)
                            nc.vector.tensor_add(
                                zsl, zsl,
                                res_sb[:, Tc, Hh * 512:(Hh + 1) * 512])
                        else:
                            tmp = tpool.tile([128, 512], f32, tag="tmp")
                            nc.vector.tensor_scalar(
                                tmp[:], psum_ys[Tc][:, :], pcol, None,
                                mybir.AluOpType.mult)
                            nc.vector.tensor_add(zsl, zsl, tmp[:])
            nc.sync.dma_start(
                out_d.ap().rearrange("(tc t) d -> t tc d", t=128), z_sb[:])
    nc.compile()
    _NC_CACHE[key] = nc
    return nc


def kernel(hidden_states, mlp_residual, probs, routing_map, w1, w2,
           _trace=False):
    hidden_states = np.ascontiguousarray(np.asarray(hidden_states, np.float32))
    mlp_residual = np.ascontiguousarray(np.asarray(mlp_residual, np.float32))
    probs = np.asarray(probs, np.float32)
    routing_map = np.asarray(routing_map, bool)
    w1 = np.asarray(w1, np.float32)
    w2 = np.asarray(w2, np.float32)

    x = hidden_states.reshape(T, H)
    res = mlp_residual.reshape(T, H)
    xt_full = np.ascontiguousarray(x.T.astype(MM_NP))  # [H, T]

    n_slots, C, windows = _plan_windows(routing_map, probs)
    # blocked w1 per expert: [NF, 128, H] with [Fc, h, Hc*128+f]
    w1blk = [np.ascontiguousarray(
        w1[e].astype(MM_NP).reshape(NH, 128, NF, 128).transpose(2, 1, 0, 3)
        .reshape(NF, 128, H)) for e in range(E)]
    w2h = w2.astype(MM_NP)

    in_maps = []
    for (experts, lst) in windows:
        n = len(lst)
        tok = np.array([v[0] for v in lst], np.int64)
        xt = np.zeros((H, C), MM_NP)
        if n:
            xt[:, :n] = xt_full[:, tok]
        pp = np.zeros((n_slots, C), np.float32)
        rr = np.zeros((C, H), np.float32)
        if n_slots == 2:
            if n:
                pp[0, :n] = [v[1] for v in lst]
                pp[1, :n] = [v[2] for v in lst]
                first = np.array([v[3] for v in lst], bool)
                rr[:n][first] = res[tok[first]]
        else:  # dense fallback: p = masked probs
            pp[:, :n] = (probs[tok] * routing_map[tok]).T
            rr[:n] = res[tok]
        w1b = np.stack([w1blk[e] for e in experts])
        w2b = np.stack([w2h[e] for e in experts])
        in_maps.append({"xt": xt, "w1b": w1b, "w2b": w2b, "pp": pp,
                        "res": rr})

    nc = _build_nc(n_slots, C)
    r = run_bass_kernel_spmd(nc, in_maps, list(range(N_CORES)),
                             trace=_trace)

    out = np.zeros((T, H), np.float32)
    ids = np.concatenate([[v[0] for v in lst] for (_, lst) in windows
                          if lst]).astype(np.int64)
    rows = np.concatenate([r.results[c]["out"][:len(windows[c][1])]
                           for c in range(N_CORES) if windows[c][1]])
    if len(np.unique(ids)) == len(ids):
        out[ids] = rows
    else:
        np.add.at(out, ids, rows)
    result = out.reshape(S, B, H)
    if _trace:
        return result, r
    return result



# revision 11
# speedup vs baseline: 2.2363x; 2.2363x over previous
"""MoE MLP (E=4, top-2 routing) Trainium2 kernel, 8 NeuronCores.

Strategy (expert-parallel x tensor-parallel): core c handles expert
e = c // 2 and FFN half = c % 2.  Each core receives ALL tokens routed to
its expert (routing_map column e), computes

    z = p_e * gelu(x @ w1[e][:, half]) @ w2[e][half, :]

for its half of the FFN dimension, and the host scatter-adds the four
partials per token (2 experts x 2 halves) plus the residual.  This is the
standard all-to-all dispatch / combine of expert parallelism, with the
dispatch/combine permutation done host-side (full-I/O contract).

Matmuls run in fp8 (e4m3) with DoubleRow perf mode: each instruction
contracts 256 elements (2 k-tiles of 128) at 0.5 cycles per output row
(4x the bf16 rate).  To stay inside the 2e-2 error budget, three of the
four quantization error sources are compensated with extra DoubleRow
passes on the residuals (hi/lo splitting), all accumulated in the same
PSUM chain:

    fc1 chain: x_hi@w1_hi + x_lo@w1_hi + x_hi@w1_lo      (12 DR steps)
    fc2 chain: a@w2_hi + a@w2_lo                         (16 DR steps)

which leaves only the gelu-output (a) quantization as an error source
(~1.3e-2 on the max-abs metric).  Weights are pre-scaled (w1*32, w2*64)
so e4m3 stays in its normal range; the scales are folded into the gelu
activation scale and the per-token prob factors.  fc2 chains are
interleaved between fc1 token-chunk groups so the PE never waits for the
activation engine to drain gelu work.
"""
import sys

import numpy as np

try:
    import concourse.bass as bass  # noqa: F401
except Exception:
    sys.path.insert(0, "/opt/trn_rl_repo")

import ml_dtypes

import concourse.bacc as bacc
import concourse.bass as bass
import concourse.mybir as mybir
import concourse.tile as tile
from concourse.bass_utils import run_bass_kernel_spmd

S, B, H, F, E = 1024, 2, 1024, 4096, 4
T = S * B
N_CORES = 8
FH = F // 2          # per-core FFN slice
NH = H // 128        # 8 h-tiles
NF1 = FH // 128      # 16 f-tiles per core (fc1 outputs)
NQ = NF1 // 2        # 8 DoubleRow contraction steps for fc2
K1 = 4               # w1_lo correction DR steps (0..4)
K2 = 8               # w2_lo correction DR steps (0..8)
WARM = 12            # PE warmup DoubleRow steps
S1 = 32.0            # w1 pre-scale (w1 ~ N(0, 1/32))
S2 = 64.0            # w2 pre-scale (w2 ~ N(0, 1/64))
F8 = mybir.dt.float8e4
F8_NP = ml_dtypes.float8_e4m3
BF16_NP = ml_dtypes.bfloat16

_NC_CACHE = {}


def _build_nc(C):
    key = (C,)
    if key in _NC_CACHE:
        return _NC_CACHE[key]
    NT = -(-C // 128)
    f32 = mybir.dt.float32
    bf16 = mybir.dt.bfloat16
    DR = mybir.MatmulPerfMode.DoubleRow
    # token chunks for fc1 (psum bank = 512 fp32)
    chunks = [(c0, min(c0 + 512, C)) for c0 in range(0, C, 512)]
    # token tiles of each chunk group, for fc1/fc2 interleaving
    gtiles = [range(c0 // 128, -(-c1 // 128)) for c0, c1 in chunks]

    nc = bacc.Bacc("TRN2", target_bir_lowering=False, debug=False,
                   num_devices=N_CORES)
    xh_d = nc.declare_dram_parameter("xh", [128, NH, C], F8, isOutput=False)
    xl_d = nc.declare_dram_parameter("xl", [128, NH, C], F8, isOutput=False)
    w1h_d = nc.declare_dram_parameter("w1h", [128, NF1 * 4 * 2, 128], F8,
                                      isOutput=False)
    w1l_d = nc.declare_dram_parameter("w1l", [128, NF1 * 4 * 2, 128], F8,
                                      isOutput=False)
    w2h_d = nc.declare_dram_parameter("w2h", [128, NQ * 2, H], F8,
                                      isOutput=False)
    w2l_d = nc.declare_dram_parameter("w2l", [128, NQ * 2, H], F8,
                                      isOutput=False)
    pp_d = nc.declare_dram_parameter("pp", [128, NT], f32, isOutput=False)
    out_d = nc.declare_dram_parameter("out", [128, NT, H], bf16,
                                      isOutput=True)

    with tile.TileContext(nc) as tc:
        with (
            tc.tile_pool(name="resident", bufs=1) as rpool,
            tc.tile_pool(name="pa", bufs=4, space="PSUM") as papool,
            tc.tile_pool(name="py", bufs=3, space="PSUM") as pypool,
        ):
            # PE warmup: keep the tensor engine continuously busy from t~0
            # so its p-state ramp (slow first 3us) completes during the
            # initial DMA window instead of during real work.
            wm_w = rpool.tile([128, 2, 128], F8, tag="wmw")
            wm_x = rpool.tile([128, 2, 512], F8, tag="wmx")
            nc.vector.memset(wm_w[:], 0.0)
            nc.vector.memset(wm_x[:], 0.0)
            wm_p = papool.tile([128, 512], mybir.dt.float32, tag="pa")
            for i in range(WARM):
                nc.tensor.matmul(wm_p[:, :], wm_w[:], wm_x[:],
                                 start=(i == 0), stop=(i == WARM - 1),
                                 perf_mode=DR)

            xh_sb = rpool.tile([128, NH, C], F8, tag="xh")
            xl_sb = rpool.tile([128, NH, C], F8, tag="xl")
            w1h_sb = rpool.tile([128, NF1 * 4 * 2, 128], F8, tag="w1h")
            w1l_sb = rpool.tile([128, NF1 * 4 * 2, 128], F8, tag="w1l")
            w2h_sb = rpool.tile([128, NQ * 2, H], F8, tag="w2h")
            w2l_sb = rpool.tile([128, NQ * 2, H], F8, tag="w2l")
            pp_sb = rpool.tile([128, NT], f32, tag="pp")
            a_sb = rpool.tile([128, NF1, C], F8, tag="a")
            z_sb = rpool.tile([128, NT, H], bf16, tag="z")

            # --- input DMAs, ordered so fc1 can start early ---
            nc.sync.dma_start(pp_sb[:], pp_d[:, :])
            c1_0 = chunks[0][1]
            nc.sync.dma_start(xh_sb[:, :, 0:c1_0], xh_d[:, :, 0:c1_0])
            for ft in range(4):  # first w1 quarter at single-tile grain
                nc.sync.dma_start(w1h_sb[:, ft * 8:(ft + 1) * 8, :],
                                  w1h_d[:, ft * 8:(ft + 1) * 8, :])
            nc.sync.dma_start(xl_sb[:, :, 0:c1_0], xl_d[:, :, 0:c1_0])
            if K1:
                for ft in range(4):
                    nc.sync.dma_start(w1l_sb[:, ft * 8:(ft + 1) * 8, :],
                                      w1l_d[:, ft * 8:(ft + 1) * 8, :])
            for q4 in range(1, 4):
                nc.sync.dma_start(w1h_sb[:, q4 * 32:(q4 + 1) * 32, :],
                                  w1h_d[:, q4 * 32:(q4 + 1) * 32, :])
                if K1:
                    nc.sync.dma_start(w1l_sb[:, q4 * 32:(q4 + 1) * 32, :],
                                      w1l_d[:, q4 * 32:(q4 + 1) * 32, :])
            for c0, c1 in chunks[1:]:
                nc.sync.dma_start(xh_sb[:, :, c0:c1], xh_d[:, :, c0:c1])
                nc.sync.dma_start(xl_sb[:, :, c0:c1], xl_d[:, :, c0:c1])
            nc.sync.dma_start(w2h_sb[:, 0:NQ, :], w2h_d[:, 0:NQ, :])
            nc.sync.dma_start(w2h_sb[:, NQ:2 * NQ, :], w2h_d[:, NQ:2 * NQ, :])
            if K2:
                nc.sync.dma_start(w2l_sb[:, 0:NQ, :], w2l_d[:, 0:NQ, :])
                nc.sync.dma_start(w2l_sb[:, NQ:2 * NQ, :],
                                  w2l_d[:, NQ:2 * NQ, :])

            def fc1_group(g):
                c0, c1 = chunks[g]
                for ft in range(NF1):
                    pa = papool.tile([128, c1 - c0], f32, tag="pa")
                    nsteps = 8 + K1
                    step = 0
                    for j in range(4):  # x_hi @ w1_hi
                        nc.tensor.matmul(
                            pa[:, :],
                            w1h_sb[:, (ft * 4 + j) * 2:(ft * 4 + j) * 2 + 2, :],
                            xh_sb[:, 2 * j:2 * j + 2, c0:c1],
                            start=(step == 0), stop=(step == nsteps - 1),
                            perf_mode=DR)
                        step += 1
                    for j in range(4):  # x_lo @ w1_hi
                        nc.tensor.matmul(
                            pa[:, :],
                            w1h_sb[:, (ft * 4 + j) * 2:(ft * 4 + j) * 2 + 2, :],
                            xl_sb[:, 2 * j:2 * j + 2, c0:c1],
                            start=(step == 0), stop=(step == nsteps - 1),
                            perf_mode=DR)
                        step += 1
                    for j in range(K1):  # x_hi @ w1_lo
                        nc.tensor.matmul(
                            pa[:, :],
                            w1l_sb[:, (ft * 4 + j) * 2:(ft * 4 + j) * 2 + 2, :],
                            xh_sb[:, 2 * j:2 * j + 2, c0:c1],
                            start=(step == 0), stop=(step == nsteps - 1),
                            perf_mode=DR)
                        step += 1
                    nc.scalar.activation(
                        a_sb[:, ft, c0:c1], pa[:, :],
                        mybir.ActivationFunctionType.Gelu, scale=1.0 / S1)

            def fc2_group(g):
                for tt in gtiles[g]:
                    t0, t1 = tt * 128, min((tt + 1) * 128, C)
                    for hc in range(2):
                        py = pypool.tile([128, 512], f32, tag="py")
                        nsteps = NQ + K2
                        step = 0
                        for q in range(NQ):  # a @ w2_hi
                            nc.tensor.matmul(
                                py[0:t1 - t0, :],
                                a_sb[:, 2 * q:2 * q + 2, t0:t1],
                                w2h_sb[:, 2 * q:2 * q + 2,
                                       hc * 512:(hc + 1) * 512],
                                start=(step == 0), stop=(step == nsteps - 1),
                                perf_mode=DR)
                            step += 1
                        for q in range(K2):  # a @ w2_lo
                            nc.tensor.matmul(
                                py[0:t1 - t0, :],
                                a_sb[:, 2 * q:2 * q + 2, t0:t1],
                                w2l_sb[:, 2 * q:2 * q + 2,
                                       hc * 512:(hc + 1) * 512],
                                start=(step == 0), stop=(step == nsteps - 1),
                                perf_mode=DR)
                            step += 1
                        nc.vector.tensor_scalar(
                            z_sb[:, tt, hc * 512:(hc + 1) * 512], py[:, :],
                            pp_sb[:, tt:tt + 1], None, mybir.AluOpType.mult)
                    nc.sync.dma_start(out_d[:, tt, :], z_sb[:, tt, :])

            # interleave: fc2 of chunk group g runs while fc1 of group g+1
            # feeds the activation engine, keeping the PE busy throughout
            ngroups = len(chunks)
            fc1_group(0)
            for g in range(1, ngroups):
                fc1_group(g)
                fc2_group(g - 1)
            fc2_group(ngroups - 1)
    nc.compile()
    _NC_CACHE[key] = nc
    return nc


def _q8(v):
    return np.asarray(v).astype(F8_NP)


def kernel(hidden_states, mlp_residual, probs, routing_map, w1, w2,
           _trace=False):
    hidden_states = np.ascontiguousarray(np.asarray(hidden_states, np.float32))
    mlp_residual = np.asarray(mlp_residual, np.float32)
    probs = np.asarray(probs, np.float32)
    routing_map = np.asarray(routing_map, bool)
    w1 = np.asarray(w1, np.float32)
    w2 = np.asarray(w2, np.float32)

    x = hidden_states.reshape(T, H)
    idx = [np.nonzero(routing_map[:, e])[0] for e in range(E)]
    # round up to 64: dual-fp8 LdWeights rejects odd/unaligned tile widths
    C = max(128, -(-max(len(i) for i in idx) // 64) * 64)
    NT = -(-C // 128)

    in_maps = []
    for c in range(N_CORES):
        e, half = c // 2, c % 2
        ids, n = idx[e], len(idx[e])
        # x^T blocked: xt[p, ht, col] = x[ids[col], ht*128 + p]
        xe = x[ids].T.reshape(NH, 128, n).transpose(1, 0, 2)
        xh = np.zeros((128, NH, C), F8_NP)
        xl = np.zeros((128, NH, C), F8_NP)
        xh[:, :, :n] = _q8(xe)
        xl[:, :, :n] = _q8(xe - xh[:, :, :n].astype(np.float32))
        # w1 half, DoubleRow blocked: [p, (ft, j, i), f],  hh = (2j+i)*128+p
        w1s = w1[e][:, half * FH:(half + 1) * FH] * S1
        w1hq = _q8(w1s)
        w1lq = _q8(w1s - w1hq.astype(np.float32))

        def blk1(a):
            return np.ascontiguousarray(
                a.reshape(4, 2, 128, NF1, 128).transpose(2, 3, 0, 1, 4)
                .reshape(128, NF1 * 4 * 2, 128))
        # w2 half, DoubleRow blocked: [p, (q, i), h],  ff = (2q+i)*128+p
        w2s = w2[e][half * FH:(half + 1) * FH, :] * S2
        w2hq = _q8(w2s)
        w2lq = _q8(w2s - w2hq.astype(np.float32))

        def blk2(a):
            return np.ascontiguousarray(
                a.reshape(NQ, 2, 128, H).transpose(2, 0, 1, 3)
                .reshape(128, NQ * 2, H))
        pp_flat = np.zeros(128 * NT, np.float32)
        pp_flat[:n] = probs[ids, e] / S2
        pp = np.ascontiguousarray(pp_flat.reshape(NT, 128).T)
        in_maps.append({"xh": xh, "xl": xl, "w1h": blk1(w1hq),
                        "w1l": blk1(w1lq), "w2h": blk2(w2hq),
                        "w2l": blk2(w2lq), "pp": pp})

    nc = _build_nc(C)
    r = run_bass_kernel_spmd(nc, in_maps, list(range(N_CORES)),
                             trace=_trace)

    out = mlp_residual.reshape(T, H).copy()
    for e in range(E):
        ids, n = idx[e], len(idx[e])
        if n == 0:
            continue
        z = (np.asarray(r.results[2 * e]["out"], np.float32)
             + np.asarray(r.results[2 * e + 1]["out"], np.float32))
        out[ids] += z.transpose(1, 0, 2).reshape(NT * 128, H)[:n]
    result = out.reshape(S, B, H)
    if _trace:
        return result, r
    return result


# revision 29
# speedup vs baseline: 2.6365x; 1.1790x over previous
"""MoE MLP (E=4, top-2 routing) Trainium2 kernel, 8 NeuronCores.

Strategy (expert-parallel x tensor-parallel): core c handles expert
e = c // 2 and FFN half = c % 2.  Each core receives ALL tokens routed to
its expert (routing_map column e), computes

    z = p_e * gelu(x @ w1[e][:, half]) @ w2[e][half, :]

for its half of the FFN dimension, and the host scatter-adds the four
partials per token (2 experts x 2 halves) plus the residual.  This is the
standard all-to-all dispatch / combine of expert parallelism, with the
dispatch/combine permutation done host-side (full-I/O contract).

Matmuls run in fp8 (e4m3) with DoubleRow perf mode: each instruction
contracts 256 elements (2 k-tiles of 128) at 0.5 cycles per output row
(4x the bf16 rate).  To stay inside the 2e-2 error budget, three of the
four quantization error sources are compensated with extra DoubleRow
passes on the residuals (hi/lo splitting), all accumulated in the same
PSUM chain:

    fc1 chain: x_hi@w1_hi + x_lo@w1_hi + x_hi@w1_lo      (12 DR steps)
    fc2 chain: a@w2_hi + a@w2_lo                         (16 DR steps)

which leaves only the gelu-output (a) quantization as an error source
(~1.3e-2 on the max-abs metric).  Weights are pre-scaled (w1*32, w2*64)
so e4m3 stays in its normal range; the scales are folded into the gelu
activation scale and the per-token prob factors.  fc2 chains are
interleaved between fc1 token-chunk groups so the PE never waits for the
activation engine to drain gelu work.
"""
import sys

import numpy as np

try:
    import concourse.bass as bass  # noqa: F401
except Exception:
    sys.path.insert(0, "/opt/trn_rl_repo")

import ml_dtypes

import concourse.bacc as bacc
import concourse.bass as bass
import concourse.mybir as mybir
import concourse.tile as tile
from concourse.bass_utils import run_bass_kernel_spmd

S, B, H, F, E = 1024, 2, 1024, 4096, 4
T = S * B
N_CORES = 8
FH = F // 2          # per-core FFN slice
NH = H // 128        # 8 h-tiles
NF1 = FH // 128      # 16 f-tiles per core (fc1 outputs)
NQ = NF1 // 2        # 8 DoubleRow contraction steps for fc2
K1 = 4               # w1_lo correction DR steps (0..4)
K2 = 2               # w2_lo correction DR steps (0..8)
WARM = 70            # PE warmup DoubleRow steps
FIRSTC = 512         # first token-chunk width
S1 = 32.0            # w1 pre-scale (w1 ~ N(0, 1/32))
S2 = 64.0            # w2 pre-scale (w2 ~ N(0, 1/64))
F8 = mybir.dt.float8e4
F8_NP = ml_dtypes.float8_e4m3
BF16_NP = ml_dtypes.bfloat16

_NC_CACHE = {}


def _build_nc(C):
    key = (C,)
    if key in _NC_CACHE:
        return _NC_CACHE[key]
    NT = -(-C // 128)
    f32 = mybir.dt.float32
    bf16 = mybir.dt.bfloat16
    DR = mybir.MatmulPerfMode.DoubleRow
    # token chunks for fc1 (psum bank = 512 fp32); a small first chunk
    # lets compute start earlier behind the initial DMAs
    bounds = [0, FIRSTC]
    while bounds[-1] < C:
        bounds.append(min(bounds[-1] + 512, C))
    chunks = list(zip(bounds, bounds[1:]))

    nc = bacc.Bacc("TRN2", target_bir_lowering=False, debug=False,
                   num_devices=N_CORES)
    xh_d = nc.declare_dram_parameter("xh", [128, NH, C], F8, isOutput=False)
    xl_d = nc.declare_dram_parameter("xl", [128, NH, C], F8, isOutput=False)
    w1h_d = nc.declare_dram_parameter("w1h", [128, NF1 * 4 * 2, 128], F8,
                                      isOutput=False)
    w1l_d = nc.declare_dram_parameter("w1l", [128, NF1 * 4 * 2, 128], F8,
                                      isOutput=False)
    w2h_d = nc.declare_dram_parameter("w2h", [128, NQ * 2, H], F8,
                                      isOutput=False)
    w2l_d = nc.declare_dram_parameter("w2l", [128, NQ * 2, H], F8,
                                      isOutput=False)
    pp_d = nc.declare_dram_parameter("pp", [128, C], f32, isOutput=False)
    out_d = nc.declare_dram_parameter("out", [128, NH, C], bf16,
                                      isOutput=True)

    with tile.TileContext(nc) as tc:
        with (
            tc.tile_pool(name="resident", bufs=1) as rpool,
            tc.tile_pool(name="pa", bufs=4, space="PSUM") as papool,
            tc.tile_pool(name="py", bufs=4, space="PSUM") as pypool,
        ):
            # PE warmup: keep the tensor engine continuously busy from t~0
            # so its p-state ramp (slow first 3us) completes during the
            # initial DMA window instead of during real work.
            wm_w = rpool.tile([128, 2, 128], F8, tag="wmw")
            nc.vector.memset(wm_w[:], 0.0)
            wm_p = papool.tile([128, 512], mybir.dt.float32, tag="pa")
            for i in range(WARM):
                nc.tensor.matmul(wm_p[:, 0:128], wm_w[:], wm_w[:],
                                 start=(i == 0), stop=(i == WARM - 1),
                                 perf_mode=DR)

            xh_sb = rpool.tile([128, NH, C], F8, tag="xh")
            xl_sb = rpool.tile([128, NH, C], F8, tag="xl")
            w1h_sb = rpool.tile([128, NF1 * 4 * 2, 128], F8, tag="w1h")
            w1l_sb = rpool.tile([128, NF1 * 4 * 2, 128], F8, tag="w1l")
            w2h_sb = rpool.tile([128, NQ * 2, H], F8, tag="w2h")
            w2l_sb = rpool.tile([128, NQ * 2, H], F8, tag="w2l")
            pp_sb = rpool.tile([128, C], f32, tag="pp")
            a_sb = rpool.tile([128, NF1, C], F8, tag="a")
            z_sb = rpool.tile([128, NH, C], bf16, tag="z")

            # --- input DMAs, ordered so fc1 can start early ---
            c1_0 = chunks[0][1]
            nc.sync.dma_start(xh_sb[:, :, 0:c1_0], xh_d[:, :, 0:c1_0])
            nc.sync.dma_start(w1h_sb[:, 0:8, :], w1h_d[:, 0:8, :])
            nc.sync.dma_start(xl_sb[:, :, 0:c1_0], xl_d[:, :, 0:c1_0])
            if K1:
                nc.sync.dma_start(w1l_sb[:, 0:8, :], w1l_d[:, 0:8, :])
            for ft in range(1, 4):  # rest of first w1 quarter, per tile
                nc.sync.dma_start(w1h_sb[:, ft * 8:(ft + 1) * 8, :],
                                  w1h_d[:, ft * 8:(ft + 1) * 8, :])
                if K1:
                    nc.sync.dma_start(w1l_sb[:, ft * 8:(ft + 1) * 8, :],
                                      w1l_d[:, ft * 8:(ft + 1) * 8, :])
            for q4 in range(1, 4):
                nc.sync.dma_start(w1h_sb[:, q4 * 32:(q4 + 1) * 32, :],
                                  w1h_d[:, q4 * 32:(q4 + 1) * 32, :])
                if K1:
                    nc.sync.dma_start(w1l_sb[:, q4 * 32:(q4 + 1) * 32, :],
                                      w1l_d[:, q4 * 32:(q4 + 1) * 32, :])
            for c0, c1 in chunks[1:]:
                nc.sync.dma_start(xh_sb[:, :, c0:c1], xh_d[:, :, c0:c1])
                nc.sync.dma_start(xl_sb[:, :, c0:c1], xl_d[:, :, c0:c1])
            nc.sync.dma_start(w2h_sb[:, 0:NQ, :], w2h_d[:, 0:NQ, :])
            nc.sync.dma_start(w2h_sb[:, NQ:2 * NQ, :], w2h_d[:, NQ:2 * NQ, :])
            if K2:
                nc.sync.dma_start(w2l_sb[:, 0:NQ, :], w2l_d[:, 0:NQ, :])
                nc.sync.dma_start(w2l_sb[:, NQ:2 * NQ, :],
                                  w2l_d[:, NQ:2 * NQ, :])
            nc.sync.dma_start(pp_sb[:], pp_d[:, :])

            def fc1_group(g):
                c0, c1 = chunks[g]
                for ft in range(NF1):
                    pa = papool.tile([128, c1 - c0], f32, tag="pa")
                    nsteps = 8 + K1
                    step = 0
                    for j in range(4):  # x_hi @ w1_hi
                        nc.tensor.matmul(
                            pa[:, :],
                            w1h_sb[:, (ft * 4 + j) * 2:(ft * 4 + j) * 2 + 2, :],
                            xh_sb[:, 2 * j:2 * j + 2, c0:c1],
                            start=(step == 0), stop=(step == nsteps - 1),
                            perf_mode=DR)
                        step += 1
                    for j in range(4):  # x_lo @ w1_hi
                        nc.tensor.matmul(
                            pa[:, :],
                            w1h_sb[:, (ft * 4 + j) * 2:(ft * 4 + j) * 2 + 2, :],
                            xl_sb[:, 2 * j:2 * j + 2, c0:c1],
                            start=(step == 0), stop=(step == nsteps - 1),
                            perf_mode=DR)
                        step += 1
                    for j in range(K1):  # x_hi @ w1_lo
                        nc.tensor.matmul(
                            pa[:, :],
                            w1l_sb[:, (ft * 4 + j) * 2:(ft * 4 + j) * 2 + 2, :],
                            xh_sb[:, 2 * j:2 * j + 2, c0:c1],
                            start=(step == 0), stop=(step == nsteps - 1),
                            perf_mode=DR)
                        step += 1
                    nc.scalar.activation(
                        a_sb[:, ft, c0:c1], pa[:, :],
                        mybir.ActivationFunctionType.Gelu, scale=1.0 / S1)

            def fc2_group(g):
                c0, c1 = chunks[g]
                for ht in range(NH):
                    py = pypool.tile([128, c1 - c0], f32, tag="py")
                    nsteps = NQ + K2
                    step = 0
                    for q in range(NQ):  # w2_hi.T @ a
                        nc.tensor.matmul(
                            py[:, :],
                            w2h_sb[:, 2 * q:2 * q + 2,
                                   ht * 128:(ht + 1) * 128],
                            a_sb[:, 2 * q:2 * q + 2, c0:c1],
                            start=(step == 0), stop=(step == nsteps - 1),
                            perf_mode=DR)
                        step += 1
                    for q in range(K2):  # w2_lo.T @ a
                        nc.tensor.matmul(
                            py[:, :],
                            w2l_sb[:, 2 * q:2 * q + 2,
                                   ht * 128:(ht + 1) * 128],
                            a_sb[:, 2 * q:2 * q + 2, c0:c1],
                            start=(step == 0), stop=(step == nsteps - 1),
                            perf_mode=DR)
                        step += 1
                    nc.vector.tensor_tensor(
                        z_sb[:, ht, c0:c1], py[:, :], pp_sb[:, c0:c1],
                        mybir.AluOpType.mult)
                    # z out in half-groups so transfers overlap compute;
                    # one DMA for narrow groups (HWDGE issue cost dominates)
                    if c1 - c0 > 128 and ht == 3:
                        nc.sync.dma_start(out_d[:, 0:4, c0:c1],
                                          z_sb[:, 0:4, c0:c1])
                    elif ht == NH - 1:
                        h0 = 4 if c1 - c0 > 128 else 0
                        nc.sync.dma_start(out_d[:, h0:NH, c0:c1],
                                          z_sb[:, h0:NH, c0:c1])

            # interleave: fc2 of chunk group g runs while fc1 of group g+1
            # feeds the activation engine, keeping the PE busy throughout
            ngroups = len(chunks)
            fc1_group(0)
            for g in range(1, ngroups):
                fc1_group(g)
                fc2_group(g - 1)
            fc2_group(ngroups - 1)
    nc.compile()
    _NC_CACHE[key] = nc
    return nc


def _q8(v):
    return np.asarray(v).astype(F8_NP)


def kernel(hidden_states, mlp_residual, probs, routing_map, w1, w2,
           _trace=False):
    hidden_states = np.ascontiguousarray(np.asarray(hidden_states, np.float32))
    mlp_residual = np.asarray(mlp_residual, np.float32)
    probs = np.asarray(probs, np.float32)
    routing_map = np.asarray(routing_map, bool)
    w1 = np.asarray(w1, np.float32)
    w2 = np.asarray(w2, np.float32)

    x = hidden_states.reshape(T, H)
    idx = [np.nonzero(routing_map[:, e])[0] for e in range(E)]
    # round up to 64: dual-fp8 LdWeights rejects odd/unaligned tile widths
    C = max(128, -(-max(len(i) for i in idx) // 64) * 64)
    NT = -(-C // 128)

    in_maps = []
    for c in range(N_CORES):
        e, half = c // 2, c % 2
        ids, n = idx[e], len(idx[e])
        # x^T blocked: xt[p, ht, col] = x[ids[col], ht*128 + p]
        xe = x[ids].T.reshape(NH, 128, n).transpose(1, 0, 2)
        xh = np.zeros((128, NH, C), F8_NP)
        xl = np.zeros((128, NH, C), F8_NP)
        xh[:, :, :n] = _q8(xe)
        xl[:, :, :n] = _q8(xe - xh[:, :, :n].astype(np.float32))
        # w1 half, DoubleRow blocked: [p, (ft, j, i), f],  hh = (2j+i)*128+p
        w1s = w1[e][:, half * FH:(half + 1) * FH] * S1
        w1hq = _q8(w1s)
        w1lq = _q8(w1s - w1hq.astype(np.float32))

        def blk1(a):
            return np.ascontiguousarray(
                a.reshape(4, 2, 128, NF1, 128).transpose(2, 3, 0, 1, 4)
                .reshape(128, NF1 * 4 * 2, 128))
        # w2 half, DoubleRow blocked: [p, (q, i), h],  ff = (2q+i)*128+p
        w2s = w2[e][half * FH:(half + 1) * FH, :] * S2
        w2hq = _q8(w2s)
        w2lq = _q8(w2s - w2hq.astype(np.float32))

        def blk2(a):
            return np.ascontiguousarray(
                a.reshape(NQ, 2, 128, H).transpose(2, 0, 1, 3)
                .reshape(128, NQ * 2, H))
        pcol = np.zeros(C, np.float32)
        pcol[:n] = probs[ids, e] / S2
        pp = np.ascontiguousarray(np.broadcast_to(pcol, (128, C)))
        in_maps.append({"xh": xh, "xl": xl, "w1h": blk1(w1hq),
                        "w1l": blk1(w1lq), "w2h": blk2(w2hq),
                        "w2l": blk2(w2lq), "pp": pp})

    nc = _build_nc(C)
    r = run_bass_kernel_spmd(nc, in_maps, list(range(N_CORES)),
                             trace=_trace)

    out = mlp_residual.reshape(T, H).copy()
    for e in range(E):
        ids, n = idx[e], len(idx[e])
        if n == 0:
            continue
        # z layout: [p, ht, c] with h = ht*128 + p
        z = (np.asarray(r.results[2 * e]["out"], np.float32)
             + np.asarray(r.results[2 * e + 1]["out"], np.float32))
        out[ids] += z.transpose(2, 1, 0).reshape(C, H)[:n]
    result = out.reshape(S, B, H)
    if _trace:
        return result, r
    return result


# revision 32
# speedup vs baseline: 2.8522x; 1.0818x over previous
"""MoE MLP (E=4, top-2 routing) Trainium2 kernel, 8 NeuronCores.

Strategy (expert-parallel x tensor-parallel): core c handles expert
e = c // 2 and FFN half = c % 2.  Each core receives ALL tokens routed to
its expert (routing_map column e), computes

    z = p_e * gelu(x @ w1[e][:, half]) @ w2[e][half, :]

for its half of the FFN dimension, and the host scatter-adds the four
partials per token (2 experts x 2 halves) plus the residual.  This is the
standard all-to-all dispatch / combine of expert parallelism, with the
dispatch/combine permutation done host-side (full-I/O contract).

Matmuls run in fp8 (e4m3) with DoubleRow perf mode: each instruction
contracts 256 elements (2 k-tiles of 128) at 0.5 cycles per output row
(4x the bf16 rate).  To stay inside the 2e-2 error budget, three of the
four quantization error sources are compensated with extra DoubleRow
passes on the residuals (hi/lo splitting), all accumulated in the same
PSUM chain:

    fc1 chain: x_hi@w1_hi + x_lo@w1_hi + x_hi@w1_lo      (12 DR steps)
    fc2 chain: a@w2_hi + a@w2_lo                         (16 DR steps)

which leaves only the gelu-output (a) quantization as an error source
(~1.3e-2 on the max-abs metric).  Weights are pre-scaled (w1*32, w2*64)
so e4m3 stays in its normal range; the scales are folded into the gelu
activation scale and the per-token prob factors.  fc2 chains are
interleaved between fc1 token-chunk groups so the PE never waits for the
activation engine to drain gelu work.
"""
import sys

import numpy as np

try:
    import concourse.bass as bass  # noqa: F401
except Exception:
    sys.path.insert(0, "/opt/trn_rl_repo")

import ml_dtypes

import concourse.bacc as bacc
import concourse.bass as bass
import concourse.mybir as mybir
import concourse.tile as tile
from concourse.bass_utils import run_bass_kernel_spmd

S, B, H, F, E = 1024, 2, 1024, 4096, 4
T = S * B
N_CORES = 8
FH = F // 2          # per-core FFN slice
NH = H // 128        # 8 h-tiles
NF1 = FH // 128      # 16 f-tiles per core (fc1 outputs)
NQ = NF1 // 2        # 8 DoubleRow contraction steps for fc2
K1 = 3               # w1_lo correction DR steps (0..4)
K2 = 2               # w2_lo correction DR steps (0..8)
WARM = 70            # PE warmup DoubleRow steps
FIRSTC = 512         # first token-chunk width
S1 = 32.0            # w1 pre-scale (w1 ~ N(0, 1/32))
S2 = 64.0            # w2 pre-scale (w2 ~ N(0, 1/64))
F8 = mybir.dt.float8e4
F8_NP = ml_dtypes.float8_e4m3
BF16_NP = ml_dtypes.bfloat16

_NC_CACHE = {}


def _build_nc(C):
    key = (C,)
    if key in _NC_CACHE:
        return _NC_CACHE[key]
    NT = -(-C // 128)
    f32 = mybir.dt.float32
    bf16 = mybir.dt.bfloat16
    DR = mybir.MatmulPerfMode.DoubleRow
    # token chunks for fc1 (psum bank = 512 fp32); a small first chunk
    # lets compute start earlier behind the initial DMAs
    bounds = [0, FIRSTC]
    while bounds[-1] < C:
        bounds.append(min(bounds[-1] + 512, C))
    chunks = list(zip(bounds, bounds[1:]))

    nc = bacc.Bacc("TRN2", target_bir_lowering=False, debug=False,
                   num_devices=N_CORES)
    xh_d = nc.declare_dram_parameter("xh", [128, NH, C], F8, isOutput=False)
    xl_d = nc.declare_dram_parameter("xl", [128, NH, C], F8, isOutput=False)
    w1h_d = nc.declare_dram_parameter("w1h", [128, NF1 * 4 * 2, 128], F8,
                                      isOutput=False)
    w1l_d = nc.declare_dram_parameter("w1l", [128, NF1 * 4 * 2, 128], F8,
                                      isOutput=False)
    w2h_d = nc.declare_dram_parameter("w2h", [128, NQ * 2, H], F8,
                                      isOutput=False)
    w2l_d = nc.declare_dram_parameter("w2l", [128, NQ * 2, H], F8,
                                      isOutput=False)
    pp_d = nc.declare_dram_parameter("pp", [128, C], f32, isOutput=False)
    out_d = nc.declare_dram_parameter("out", [128, NH, C], bf16,
                                      isOutput=True)

    with tile.TileContext(nc) as tc:
        with (
            tc.tile_pool(name="resident", bufs=1) as rpool,
            tc.tile_pool(name="pa", bufs=3, space="PSUM") as papool,
            tc.tile_pool(name="py", bufs=5, space="PSUM") as pypool,
        ):
            # PE warmup: keep the tensor engine continuously busy from t~0
            # so its p-state ramp (slow first 3us) completes during the
            # initial DMA window instead of during real work.
            wm_w = rpool.tile([128, 2, 128], F8, tag="wmw")
            nc.vector.memset(wm_w[:], 0.0)
            wm_p = papool.tile([128, 512], mybir.dt.float32, tag="pa")
            for i in range(WARM):
                nc.tensor.matmul(wm_p[:, 0:128], wm_w[:], wm_w[:],
                                 start=(i == 0), stop=(i == WARM - 1),
                                 perf_mode=DR)

            xh_sb = rpool.tile([128, NH, C], F8, tag="xh")
            xl_sb = rpool.tile([128, NH, C], F8, tag="xl")
            w1h_sb = rpool.tile([128, NF1 * 4 * 2, 128], F8, tag="w1h")
            w1l_sb = rpool.tile([128, NF1 * 4 * 2, 128], F8, tag="w1l")
            w2h_sb = rpool.tile([128, NQ * 2, H], F8, tag="w2h")
            w2l_sb = rpool.tile([128, NQ * 2, H], F8, tag="w2l")
            pp_sb = rpool.tile([128, C], f32, tag="pp")
            a_sb = rpool.tile([128, NF1, C], F8, tag="a")
            z_sb = rpool.tile([128, NH, C], bf16, tag="z")

            # --- input DMAs, ordered so fc1 can start early ---
            c1_0 = chunks[0][1]
            nc.sync.dma_start(xh_sb[:, :, 0:c1_0], xh_d[:, :, 0:c1_0])
            nc.sync.dma_start(w1h_sb[:, 0:8, :], w1h_d[:, 0:8, :])
            nc.sync.dma_start(xl_sb[:, :, 0:c1_0], xl_d[:, :, 0:c1_0])
            if K1:
                nc.sync.dma_start(w1l_sb[:, 0:8, :], w1l_d[:, 0:8, :])
            for ft in range(1, 4):  # rest of first w1 quarter, per tile
                nc.sync.dma_start(w1h_sb[:, ft * 8:(ft + 1) * 8, :],
                                  w1h_d[:, ft * 8:(ft + 1) * 8, :])
                if K1:
                    nc.sync.dma_start(w1l_sb[:, ft * 8:(ft + 1) * 8, :],
                                      w1l_d[:, ft * 8:(ft + 1) * 8, :])
            for q4 in range(1, 4):
                nc.sync.dma_start(w1h_sb[:, q4 * 32:(q4 + 1) * 32, :],
                                  w1h_d[:, q4 * 32:(q4 + 1) * 32, :])
                if K1:
                    nc.sync.dma_start(w1l_sb[:, q4 * 32:(q4 + 1) * 32, :],
                                      w1l_d[:, q4 * 32:(q4 + 1) * 32, :])
            for c0, c1 in chunks[1:]:
                nc.sync.dma_start(xh_sb[:, :, c0:c1], xh_d[:, :, c0:c1])
                nc.sync.dma_start(xl_sb[:, :, c0:c1], xl_d[:, :, c0:c1])
            nc.sync.dma_start(w2h_sb[:, 0:NQ, :], w2h_d[:, 0:NQ, :])
            nc.sync.dma_start(w2h_sb[:, NQ:2 * NQ, :], w2h_d[:, NQ:2 * NQ, :])
            if K2:
                nc.sync.dma_start(w2l_sb[:, 0:NQ, :], w2l_d[:, 0:NQ, :])
                nc.sync.dma_start(w2l_sb[:, NQ:2 * NQ, :],
                                  w2l_d[:, NQ:2 * NQ, :])
            nc.sync.dma_start(pp_sb[:], pp_d[:, :])

            def fc1_group(g):
                c0, c1 = chunks[g]
                for ft in range(NF1):
                    pa = papool.tile([128, c1 - c0], f32, tag="pa")
                    nsteps = 8 + K1
                    step = 0
                    for j in range(4):  # x_hi @ w1_hi
                        nc.tensor.matmul(
                            pa[:, :],
                            w1h_sb[:, (ft * 4 + j) * 2:(ft * 4 + j) * 2 + 2, :],
                            xh_sb[:, 2 * j:2 * j + 2, c0:c1],
                            start=(step == 0), stop=(step == nsteps - 1),
                            perf_mode=DR)
                        step += 1
                    for j in range(4):  # x_lo @ w1_hi
                        nc.tensor.matmul(
                            pa[:, :],
                            w1h_sb[:, (ft * 4 + j) * 2:(ft * 4 + j) * 2 + 2, :],
                            xl_sb[:, 2 * j:2 * j + 2, c0:c1],
                            start=(step == 0), stop=(step == nsteps - 1),
                            perf_mode=DR)
                        step += 1
                    for j in range(K1):  # x_hi @ w1_lo
                        nc.tensor.matmul(
                            pa[:, :],
                            w1l_sb[:, (ft * 4 + j) * 2:(ft * 4 + j) * 2 + 2, :],
                            xh_sb[:, 2 * j:2 * j + 2, c0:c1],
                            start=(step == 0), stop=(step == nsteps - 1),
                            perf_mode=DR)
                        step += 1
                    nc.scalar.activation(
                        a_sb[:, ft, c0:c1], pa[:, :],
                        mybir.ActivationFunctionType.Gelu, scale=1.0 / S1)

            def fc2_group(g):
                c0, c1 = chunks[g]
                for ht in range(NH):
                    py = pypool.tile([128, c1 - c0], f32, tag="py")
                    nsteps = NQ + K2
                    step = 0
                    for q in range(NQ):  # w2_hi.T @ a
                        nc.tensor.matmul(
                            py[:, :],
                            w2h_sb[:, 2 * q:2 * q + 2,
                                   ht * 128:(ht + 1) * 128],
                            a_sb[:, 2 * q:2 * q + 2, c0:c1],
                            start=(step == 0), stop=(step == nsteps - 1),
                            perf_mode=DR)
                        step += 1
                    for q in range(K2):  # w2_lo.T @ a
                        nc.tensor.matmul(
                            py[:, :],
                            w2l_sb[:, 2 * q:2 * q + 2,
                                   ht * 128:(ht + 1) * 128],
                            a_sb[:, 2 * q:2 * q + 2, c0:c1],
                            start=(step == 0), stop=(step == nsteps - 1),
                            perf_mode=DR)
                        step += 1
                    nc.vector.tensor_tensor(
                        z_sb[:, ht, c0:c1], py[:, :], pp_sb[:, c0:c1],
                        mybir.AluOpType.mult)
                    # z out in quarter-groups so transfers overlap compute;
                    # one DMA for narrow groups (HWDGE issue cost dominates)
                    if c1 - c0 > 128:
                        if ht % 2 == 1:
                            nc.sync.dma_start(out_d[:, ht - 1:ht + 1, c0:c1],
                                              z_sb[:, ht - 1:ht + 1, c0:c1])
                    elif ht == NH - 1:
                        nc.sync.dma_start(out_d[:, :, c0:c1],
                                          z_sb[:, :, c0:c1])

            # interleave: fc2 of chunk group g runs while fc1 of group g+1
            # feeds the activation engine, keeping the PE busy throughout
            ngroups = len(chunks)
            fc1_group(0)
            for g in range(1, ngroups):
                fc1_group(g)
                fc2_group(g - 1)
            fc2_group(ngroups - 1)
    nc.compile()
    _NC_CACHE[key] = nc
    return nc


def _q8(v):
    return np.asarray(v).astype(F8_NP)


def kernel(hidden_states, mlp_residual, probs, routing_map, w1, w2,
           _trace=False):
    hidden_states = np.ascontiguousarray(np.asarray(hidden_states, np.float32))
    mlp_residual = np.asarray(mlp_residual, np.float32)
    probs = np.asarray(probs, np.float32)
    routing_map = np.asarray(routing_map, bool)
    w1 = np.asarray(w1, np.float32)
    w2 = np.asarray(w2, np.float32)

    x = hidden_states.reshape(T, H)
    idx = [np.nonzero(routing_map[:, e])[0] for e in range(E)]
    C = max(128, max(len(i) for i in idx))
    NT = -(-C // 128)

    in_maps = []
    for c in range(N_CORES):
        e, half = c // 2, c % 2
        ids, n = idx[e], len(idx[e])
        # x^T blocked: xt[p, ht, col] = x[ids[col], ht*128 + p]
        xe = x[ids].T.reshape(NH, 128, n).transpose(1, 0, 2)
        xh = np.zeros((128, NH, C), F8_NP)
        xl = np.zeros((128, NH, C), F8_NP)
        xh[:, :, :n] = _q8(xe)
        xl[:, :, :n] = _q8(xe - xh[:, :, :n].astype(np.float32))
        # w1 half, DoubleRow blocked: [p, (ft, j, i), f],  hh = (2j+i)*128+p
        w1s = w1[e][:, half * FH:(half + 1) * FH] * S1
        w1hq = _q8(w1s)
        w1lq = _q8(w1s - w1hq.astype(np.float32))

        def blk1(a):
            return np.ascontiguousarray(
                a.reshape(4, 2, 128, NF1, 128).transpose(2, 3, 0, 1, 4)
                .reshape(128, NF1 * 4 * 2, 128))
        # w2 half, DoubleRow blocked: [p, (q, i), h],  ff = (2q+i)*128+p
        w2s = w2[e][half * FH:(half + 1) * FH, :] * S2
        w2hq = _q8(w2s)
        w2lq = _q8(w2s - w2hq.astype(np.float32))

        def blk2(a):
            return np.ascontiguousarray(
                a.reshape(NQ, 2, 128, H).transpose(2, 0, 1, 3)
                .reshape(128, NQ * 2, H))
        pcol = np.zeros(C, np.float32)
        pcol[:n] = probs[ids, e] / S2
        pp = np.ascontiguousarray(np.broadcast_to(pcol, (128, C)))
        in_maps.append({"xh": xh, "xl": xl, "w1h": blk1(w1hq),
                        "w1l": blk1(w1lq), "w2h": blk2(w2hq),
                        "w2l": blk2(w2lq), "pp": pp})

    nc = _build_nc(C)
    r = run_bass_kernel_spmd(nc, in_maps, list(range(N_CORES)),
                             trace=_trace)

    out = mlp_residual.reshape(T, H).copy()
    for e in range(E):
        ids, n = idx[e], len(idx[e])
        if n == 0:
            continue
        # z layout: [p, ht, c] with h = ht*128 + p
        z = (np.asarray(r.results[2 * e]["out"], np.float32)
             + np.asarray(r.results[2 * e + 1]["out"], np.float32))
        out[ids] += z.transpose(2, 1, 0).reshape(C, H)[:n]
    result = out.reshape(S, B, H)
    if _trace:
        return result, r
    return result
